# revision 1
# baseline (speedup 1.0000x reference)
"""Distributed matvec kernel for nn_CubicalModel_ISM.

Computes Xp = I @ p, Yp = J @ p with I, J: [784, 50000], p: [50000], then
gathers tiny [50, 2] persistence diagrams from the 28x28 reshapes.

Strategy (8 NeuronCores):
  - Shard the contraction dim P=50000 column-wise across 8 cores
    (6400 = 50*128 per core, zero-padded at the tail).
  - Host-side: transpose each shard to [K, 784], split fp32 into bf16
    hi + bf16 lo planes (same total bytes as fp32, so the memory
    roofline is unchanged, but the PE runs at bf16 rate instead of the
    4x-slower fp32 mode), and pack pairs of 128-row k-subtiles
    side-by-side so each DMA moves a fully contiguous [128 x 3136B]
    block (802 KB). p is split the same way; products
    hi*hi + hi*lo + lo*hi are accumulated in fp32 PSUM, recovering
    fp32-level precision (dropped lo*lo term is ~2^-18 relative).
  - Raw Bass (no Tile): this walrus build supports only ONE sync-wait
    per DMA instruction. Each DMA carries exactly one embedded wait --
    on its own round-robin lane's predecessor -- which strictly orders
    every lane's semaphore updates (race-free counts). All other waits
    are standalone engine wait_ge ops.
  - Host: sum the 8 cores' partials (the "all-reduce"), reshape, gather.
"""

import numpy as np
import ml_dtypes

import concourse.bass as bass
import concourse.mybir as mybir
from concourse.bass_utils import run_bass_kernel_spmd

N_CORES = 8
P_FULL = 50000
H = W = 28
M = H * W  # 784
KT = 50  # k-subtiles (of 128) per core
K_PER = KT * 128  # 6400
NT = KT // 2  # 25 double-tiles per plane
M2 = 2 * M  # 1568 bf16 cols per double-tile
NHALF = 392  # 784 / 2, per-PSUM-bank output chunk

BF16 = ml_dtypes.bfloat16
F32 = np.float32

B = 12  # double-tile buffers per plane (4 planes x B x 3136B/partition)
N_LANES = 12  # round-robin lanes on the SP HWDGE queue


def build_nc() -> bass.Bass:
    f32 = mybir.dt.float32
    bf16 = mybir.dt.bfloat16
    nc = bass.Bass("TRN2")
    pw_d = nc.dram_tensor("pw", [128, 2 * KT], bf16, kind="ExternalInput")
    planes_d = {
        name: nc.dram_tensor(name, [NT * 128, M2], bf16, kind="ExternalInput")
        for name in ("ihi", "ilo", "jhi", "jlo")
    }
    out_d = nc.dram_tensor("out", [6, M], f32, kind="ExternalOutput")

    tiled = {
        name: t[:, :].rearrange("(n p) m -> n p m", p=128)
        for name, t in planes_d.items()
    }

    from contextlib import ExitStack

    with ExitStack() as stk:
        pw_sb = stk.enter_context(nc.sbuf_tensor("pw_sb", [128, 2 * KT], bf16))
        streams = {
            name: stk.enter_context(
                nc.sbuf_tensor(f"s_{name}", [128, B * M2], bf16)
            )
            for name in ("ihi", "ilo", "jhi", "jlo")
        }
        o_ih = stk.enter_context(nc.sbuf_tensor("o_ih", [2, M], f32))
        o_il = stk.enter_context(nc.sbuf_tensor("o_il", [1, M], f32))
        o_jh = stk.enter_context(nc.sbuf_tensor("o_jh", [2, M], f32))
        o_jl = stk.enter_context(nc.sbuf_tensor("o_jl", [1, M], f32))
        ps = {
            ("i", "h"): tuple(
                stk.enter_context(nc.psum_tensor(f"ps_ih{c}", [2, NHALF], f32))
                for c in range(2)
            ),
            ("i", "l"): tuple(
                stk.enter_context(nc.psum_tensor(f"ps_il{c}", [1, NHALF], f32))
                for c in range(2)
            ),
            ("j", "h"): tuple(
                stk.enter_context(nc.psum_tensor(f"ps_jh{c}", [2, NHALF], f32))
                for c in range(2)
            ),
            ("j", "l"): tuple(
                stk.enter_context(nc.psum_tensor(f"ps_jl{c}", [1, NHALF], f32))
                for c in range(2)
            ),
        }
        sp_lanes = [
            stk.enter_context(nc.semaphore(f"spl{q}"))
            for q in range(N_LANES)
        ]
        pe_sem = stk.enter_context(nc.semaphore("pe_sem"))
        pe_i_sem = stk.enter_context(nc.semaphore("pe_i_sem"))
        dve_sem = stk.enter_context(nc.semaphore("dve_sem"))
        block = stk.enter_context(nc.Block(no_gpsimd_drain=True))

        outs = {("i", "h"): o_ih, ("i", "l"): o_il,
                ("j", "h"): o_jh, ("j", "l"): o_jl}

        def slot_cols(n):
            s = (n % B) * M2
            return slice(s, s + M2)

        # Per-queue round-robin lane bookkeeping (see module docstring).
        dma_records = {}

        def make_issuer(lanes):
            state = {"k": 0, "counts": [0] * len(lanes)}

            def issue(eng, dst, src, record_key):
                q = state["k"] % len(lanes)
                state["k"] += 1
                prev = state["counts"][q]
                ins = eng.dma_start(dst, src).then_inc(lanes[q], 16)
                if prev > 0:
                    ins.wait_op(lanes[q], 16 * prev, "sem-ge")
                state["counts"][q] = prev + 1
                dma_records.setdefault(record_key, []).append(
                    (lanes[q], 16 * (prev + 1))
                )

            return issue

        issue_sp = make_issuer(sp_lanes)

        @block.sync
        def _(sync):
            issue_sp(sync, pw_sb[:, :], pw_d[:, :], ("pw",))
            for n in range(NT):
                if n >= B:
                    # slot n%B was last used by double-tile n-B; wait until
                    # the PE consumed it (pe_sem counts finished double-tiles)
                    sync.wait_ge(pe_sem, n - B + 1)
                cols = slot_cols(n)
                for name in ("ihi", "ilo", "jhi", "jlo"):
                    issue_sp(
                        sync, streams[name][:, cols], tiled[name][n, :, :],
                        ("tile", n),
                    )
            # ship I's outputs as soon as the DVE evicted them (overlaps
            # J's last matmuls + eviction), then J's
            sync.wait_ge(dve_sem, 1)
            issue_sp(sync, out_d[0:2, :], o_ih[:, :], ("out",))
            issue_sp(sync, out_d[2:3, :], o_il[:, :], ("out",))
            sync.wait_ge(dve_sem, 2)
            issue_sp(sync, out_d[3:5, :], o_jh[:, :], ("out",))
            sync.wait_ge(dve_sem, 3)
            issue_sp(sync, out_d[5:6, :], o_jl[:, :], ("out",))
            for sem, v in dma_records[("out",)]:
                sync.wait_ge(sem, v)

        @block.tensor
        def _(tensor):
            for n in range(NT):
                if n == 0:
                    for sem, v in dma_records[("pw",)]:
                        tensor.wait_ge(sem, v)
                # records 0,1 = ihi/ilo DMAs; 2,3 = jhi/jlo. Waiting per
                # plane pair lets I's matmuls run while J's planes are
                # still in flight (on the last tile, I's eviction and
                # output DMA complete before the stream ends).
                recs = dma_records[("tile", n)]
                for sem, v in recs[:2]:
                    tensor.wait_ge(sem, v)
                cols = slot_cols(n)
                last = None

                def emit(mat, two, n=n, cols=cols):
                    s = 2 * n + two  # k-subtile index
                    start = s == 0
                    stop = s == KT - 1
                    w2 = pw_sb[:, 2 * s : 2 * s + 2]  # [128, 2] (p_hi, p_lo)
                    w1 = pw_sb[:, 2 * s : 2 * s + 1]  # [128, 1] (p_hi)
                    last = None
                    for c in range(2):
                        cs = slice(
                            cols.start + two * M + c * NHALF,
                            cols.start + two * M + (c + 1) * NHALF,
                        )
                        last = nc.tensor.matmul(
                            ps[(mat, "h")][c][:, :], w2,
                            streams[f"{mat}hi"][:, cs],
                            start=start, stop=stop,
                        )
                        last = nc.tensor.matmul(
                            ps[(mat, "l")][c][:, :], w1,
                            streams[f"{mat}lo"][:, cs],
                            start=start, stop=stop,
                        )
                    return last

                for two in range(2):
                    last = emit("i", two)
                if n == NT - 1:
                    last.then_inc(pe_i_sem, 1)
                for sem, v in recs[2:]:
                    tensor.wait_ge(sem, v)
                for two in range(2):
                    last = emit("j", two)
                last.then_inc(pe_sem, 1)

        @block.vector
        def _(vector):
            # I's PSUMs close one half-tile before J's (matrix-major order
            # on the last tile) -- evict + ship them while J still runs
            vector.wait_ge(pe_i_sem, 1)
            last = None
            for hl in ("h", "l"):
                for c in range(2):
                    cs = slice(c * NHALF, (c + 1) * NHALF)
                    last = nc.vector.tensor_copy(
                        outs[("i", hl)][:, cs], ps[("i", hl)][c][:, :]
                    )
            last.then_inc(dve_sem, 1)
            vector.wait_ge(pe_sem, NT)
            for hl in ("h", "l"):
                for c in range(2):
                    cs = slice(c * NHALF, (c + 1) * NHALF)
                    last = nc.vector.tensor_copy(
                        outs[("j", hl)][:, cs], ps[("j", hl)][c][:, :]
                    )
                # ship o_jh while o_jl is still being copied
                last.then_inc(dve_sem, 1)

    return nc


_NC_CACHE = None


def get_nc() -> bass.Bass:
    global _NC_CACHE
    if _NC_CACHE is None:
        _NC_CACHE = build_nc()
    return _NC_CACHE


def _split_hi_lo(a32: np.ndarray):
    hi = a32.astype(BF16)
    lo = (a32 - hi.astype(F32)).astype(BF16)
    return hi, lo


def _pack_pairs(plane: np.ndarray) -> np.ndarray:
    """[K_PER, M] -> [NT*128, 2*M]: subtiles 2n,2n+1 side by side so one
    DMA moves a fully contiguous [128 x 3136B] block."""
    return np.ascontiguousarray(
        plane.reshape(NT, 2, 128, M).transpose(0, 2, 1, 3).reshape(NT * 128, M2)
    )


def shard_inputs(p, I, J) -> list[dict]:
    p = np.asarray(p, dtype=F32)
    I = np.asarray(I, dtype=F32)
    J = np.asarray(J, dtype=F32)

    p_pad = np.zeros(N_CORES * K_PER, dtype=F32)
    p_pad[:P_FULL] = p

    in_maps = []
    for c in range(N_CORES):
        lo_k = c * K_PER
        hi_k = min(lo_k + K_PER, P_FULL)
        kc = hi_k - lo_k

        pc = p_pad[c * K_PER : (c + 1) * K_PER]
        phi, plo = _split_hi_lo(pc)
        pw = np.zeros((128, 2 * KT), dtype=BF16)
        pw[:, 0::2] = phi.reshape(KT, 128).T
        pw[:, 1::2] = plo.reshape(KT, 128).T

        im = {"pw": pw}
        for name, mat in (("i", I), ("j", J)):
            t = np.zeros((K_PER, M), dtype=F32)
            if kc > 0:
                t[:kc] = mat[:, lo_k:hi_k].T
            hi_p, lo_p = _split_hi_lo(t)
            im[f"{name}hi"] = _pack_pairs(hi_p)
            im[f"{name}lo"] = _pack_pairs(lo_p)
        in_maps.append(im)
    return in_maps


def run(p, I, J, inds1, inds2, trace=False, **run_kwargs):
    """Returns ((dgm1, dgm2), BassKernelResults)."""
    in_maps = shard_inputs(p, I, J)
    nc = get_nc()
    res = run_bass_kernel_spmd(
        nc, in_maps, list(range(N_CORES)), trace=trace, **run_kwargs
    )
    acc = np.zeros((6, M), dtype=np.float64)
    for r in res.results:
        acc += r["out"].astype(np.float64)
    Xp = (acc[0] + acc[1] + acc[2]).astype(F32).reshape(H, W)
    Yp = (acc[3] + acc[4] + acc[5]).astype(F32).reshape(H, W)
    inds1 = np.asarray(inds1)
    inds2 = np.asarray(inds2)
    dgm1 = Xp[inds1[:, 0], inds1[:, 1]].reshape(-1, 2)
    dgm2 = Yp[inds2[:, 0], inds2[:, 1]].reshape(-1, 2)
    return (dgm1, dgm2), res


def kernel(p, I, J, inds1, inds2):
    out, _ = run(p, I, J, inds1, inds2, trace=False)
    return out



# revision 2
# speedup vs baseline: 5.4391x; 5.4391x over previous
"""Distributed selected-row matvec kernel for nn_CubicalModel_ISM.

The reference computes Xp = I @ p, Yp = J @ p (I, J: [784, 50000]) and then
gathers 100 pixels from each 28x28 image to form two [50, 2] diagrams. Only
the <=100 gathered rows of I and <=100 rows of J can influence the output, so
the device kernel computes exactly those 200 dot products (dead-code
elimination of the other ~584 rows), cutting HBM traffic ~4x.

Strategy (8 NeuronCores):
  - Host: rows = inds[:,0]*28 + inds[:,1]; Xsel = vstack(I[rows1], J[rows2])
    -> [200, 50000], rounded once to bf16 (norm-rel error ~2e-3, tolerance
    2e-2). p stays split as bf16 hi + bf16 lo (costs nothing: both p rows
    ride in the same matmul), so only the matrix rounding contributes error.
  - Shard the contraction dim P=50000 across 8 cores: 6250 each, zero-padded
    to 6400 = 50 k-subtiles of 128. Per-core plane = Xsel shard transposed to
    [6400, 200] and packed so each of 5 DMAs moves a contiguous
    [128 x 4000B] block (512 KB).
  - Device: 50 accumulating matmuls [128,2]^T @ [128,200] into one fp32 PSUM
    bank [2, 200]; DVE evicts to SBUF; one tiny output DMA [2,200] f32.
  - Raw Bass (no Tile): every DMA gets its own semaphore, no embedded waits.
  - Host: sum the 8 cores' (hi, lo) partial rows in float64, reshape to the
    two [50, 2] diagrams.
"""

import numpy as np
import ml_dtypes

import concourse.bass as bass
import concourse.mybir as mybir
from concourse.bass_utils import run_bass_kernel_spmd

N_CORES = 8
P_FULL = 50000
H = W = 28
NSEL = 200  # selected rows: 100 from I + 100 from J
K_PER = P_FULL // N_CORES  # 6250
KT = 50  # k-subtiles of 128 per core (6400 padded)
K_PAD = KT * 128  # 6400
G = 10  # k-subtiles packed per DMA tile
NT = KT // G  # 5 input tiles
TC = G * NSEL  # 2000 bf16 cols per tile

BF16 = ml_dtypes.bfloat16
F32 = np.float32


def build_nc() -> bass.Bass:
    f32 = mybir.dt.float32
    bf16 = mybir.dt.bfloat16
    nc = bass.Bass("TRN2")
    pw_d = nc.dram_tensor("pw", [128, 2 * KT], bf16, kind="ExternalInput")
    x_d = nc.dram_tensor("x", [NT * 128, TC], bf16, kind="ExternalInput")
    out_d = nc.dram_tensor("out", [2, NSEL], f32, kind="ExternalOutput")

    x_tiled = x_d[:, :].rearrange("(n p) m -> n p m", p=128)

    from contextlib import ExitStack

    with ExitStack() as stk:
        pw_sb = stk.enter_context(nc.sbuf_tensor("pw_sb", [128, 2 * KT], bf16))
        x_sb = stk.enter_context(nc.sbuf_tensor("x_sb", [128, NT * TC], bf16))
        o_sb = stk.enter_context(nc.sbuf_tensor("o_sb", [2, NSEL], f32))
        ps = stk.enter_context(nc.psum_tensor("ps", [2, NSEL], f32))
        d_pw = stk.enter_context(nc.semaphore("d_pw"))
        d_x = [stk.enter_context(nc.semaphore(f"d_x{n}")) for n in range(NT)]
        d_out = stk.enter_context(nc.semaphore("d_out"))
        pe_sem = stk.enter_context(nc.semaphore("pe_sem"))
        dve_sem = stk.enter_context(nc.semaphore("dve_sem"))
        block = stk.enter_context(nc.Block(no_gpsimd_drain=True))

        @block.sync
        def _(sync):
            sync.dma_start(pw_sb[:, :], pw_d[:, :]).then_inc(d_pw, 16)
            for n in range(NT):
                sync.dma_start(
                    x_sb[:, n * TC : (n + 1) * TC], x_tiled[n, :, :]
                ).then_inc(d_x[n], 16)
            sync.wait_ge(dve_sem, 1)
            sync.dma_start(out_d[:, :], o_sb[:, :]).then_inc(d_out, 16)
            sync.wait_ge(d_out, 16)

        @block.tensor
        def _(tensor):
            tensor.wait_ge(d_pw, 16)
            for n in range(NT):
                tensor.wait_ge(d_x[n], 16)
                for g in range(G):
                    s = n * G + g
                    ins = nc.tensor.matmul(
                        ps[:, :],
                        pw_sb[:, 2 * s : 2 * s + 2],
                        x_sb[:, n * TC + g * NSEL : n * TC + (g + 1) * NSEL],
                        start=(s == 0),
                        stop=(s == KT - 1),
                    )
            ins.then_inc(pe_sem, 1)

        @block.vector
        def _(vector):
            vector.wait_ge(pe_sem, 1)
            nc.vector.tensor_copy(o_sb[:, :], ps[:, :]).then_inc(dve_sem, 1)

    return nc


_NC_CACHE = None


def get_nc() -> bass.Bass:
    global _NC_CACHE
    if _NC_CACHE is None:
        _NC_CACHE = build_nc()
    return _NC_CACHE


def shard_inputs(p, I, J, inds1, inds2) -> list[dict]:
    p = np.asarray(p, dtype=F32)
    I = np.asarray(I, dtype=F32)
    J = np.asarray(J, dtype=F32)
    inds1 = np.asarray(inds1).astype(np.int64)
    inds2 = np.asarray(inds2).astype(np.int64)

    rows1 = inds1[:, 0] * W + inds1[:, 1]
    rows2 = inds2[:, 0] * W + inds2[:, 1]
    xsel = np.concatenate([I[rows1], J[rows2]], axis=0)  # [200, 50000]
    xsel_b = xsel.astype(BF16)

    in_maps = []
    for c in range(N_CORES):
        lo_k = c * K_PER
        pc = np.zeros(K_PAD, dtype=F32)
        pc[:K_PER] = p[lo_k : lo_k + K_PER]
        phi = pc.astype(BF16)
        plo = (pc - phi.astype(F32)).astype(BF16)
        pw = np.zeros((128, 2 * KT), dtype=BF16)
        pw[:, 0::2] = phi.reshape(KT, 128).T
        pw[:, 1::2] = plo.reshape(KT, 128).T

        plane = np.zeros((K_PAD, NSEL), dtype=BF16)
        plane[:K_PER] = xsel_b[:, lo_k : lo_k + K_PER].T
        packed = np.ascontiguousarray(
            plane.reshape(NT, G, 128, NSEL)
            .transpose(0, 2, 1, 3)
            .reshape(NT * 128, TC)
        )
        in_maps.append({"pw": pw, "x": packed})
    return in_maps


def run(p, I, J, inds1, inds2, trace=False, **run_kwargs):
    """Returns ((dgm1, dgm2), BassKernelResults)."""
    in_maps = shard_inputs(p, I, J, inds1, inds2)
    nc = get_nc()
    res = run_bass_kernel_spmd(
        nc, in_maps, list(range(N_CORES)), trace=trace, **run_kwargs
    )
    acc = np.zeros(NSEL, dtype=np.float64)
    for r in res.results:
        o = r["out"].astype(np.float64)
        acc += o[0] + o[1]
    y = acc.astype(F32)
    dgm1 = y[:100].reshape(-1, 2)
    dgm2 = y[100:].reshape(-1, 2)
    return (dgm1, dgm2), res


def kernel(p, I, J, inds1, inds2):
    out, _ = run(p, I, J, inds1, inds2, trace=False)
    return out
